# revision 2
# baseline (speedup 1.0000x reference)
"""Trainium2 Bass kernel for BinarizedInputNetwork — v2.

Contract: kernel(**inputs) takes the FULL unsharded inputs (batch 128) and
returns the FULL [128, 12] float32 softmax output. Internally shards the
batch across 8 NeuronCores (16 images each), runs one SPMD Bass program.

v2 changes vs baseline:
  - conv1 as block-diagonal K=54 fp16 matmuls (2 images per MM: the pair's
    im2col on partitions 0-26/27-53, couts at psum 0-63/64-127) x row-tiled
    pairs at partition offsets {0,64} => 4 images per PE slot. Odd images
    keep their A1 partition halves swapped (psum-half alignment), with
    partition-swapped L2 weight packs.
  - A1 on a 16-aligned pitch (80) so DoubleRow j-steps can express row
    shifts directly; one +1-col shifted block. L2 repacked: MM1 covers
    (0,0),(1,0),(0,1),(1,1); MM2 covers (0,2),(1,2),(2,2) via j-step
    +2 rows; leftover {(2,0),(2,1)} is a K=64 unit row-tile-paired across
    chunk pairs (two concurrent half-K matmuls) -> 2.5 slots/chunk.
  - A3 switched to {0,1} coding via DVE is_gt (moves L3 binarize off the
    Activation engine); thresholds folded accordingly.
  - Shifted-copy DMAs merged (A1 copy2+block, A3 block: one DMA each).
"""

import sys

sys.path.insert(0, "/opt/trn_rl_repo")

import numpy as np

import concourse.ap as apm
import concourse.bass as bass
import concourse.mybir as mybir
import concourse.bacc as bacc
import concourse.tile as tile
from concourse.bass_utils import run_bass_kernel_spmd

F32 = mybir.dt.float32
F16 = mybir.dt.float16
FP8 = mybir.dt.float8e4
AX = mybir.AxisListType
OP = mybir.AluOpType
ACT = mybir.ActivationFunctionType
DR = mybir.MatmulPerfMode.DoubleRow

N_CORES = 8
B = 16  # images per core

EPS = 1e-5

# geometry
PA = 80                  # A1/A2 row pitch (16-aligned)
S1A = PA * 67            # A1 area incl guard row (5360)
BO1 = 5376               # A1 +1-col block offset (16-aligned)
S2A = PA * 66            # A2 interior area (5280)
BO2 = 5376               # A2 +1-col block offset
P3A = 48                 # A3 row pitch (34 rows x 48)
S3A = 34 * P3A           # 1632
BO3 = 1632               # A3 +1-col block offset (16-aligned)
NPOS = 1024              # 32*32 valid positions for L5/GAP

_CACHE = {}


def _ap(base2d, off, dims):
    """Custom AP over an SBUF tile slice: base partition dim + free dims
    (supports overlapping patterns rearrange can't express)."""
    return apm.AP(tensor=base2d.tensor, offset=base2d.offset + off,
                  ap=[list(base2d.ap[0])] + [list(d) for d in dims])


def _build(reps=1):
    key = f"nc{reps}"
    if key in _CACHE:
        return _CACHE

    nc = bacc.Bacc("TRN2", target_bir_lowering=False, debug=False,
                   num_devices=N_CORES)

    # ---- DRAM I/O ----
    # host-side im2col, fp16 split-precision, packed in image PAIRS:
    # [8, 54, 4096]: rows 0-26 = even img taps (hi,hi,lo*64), 27-53 = odd.
    dX = nc.dram_tensor("x", [B // 2, 54, 4096], F16,
                        kind="ExternalInput").ap()
    dW1T = nc.dram_tensor("w1t", [128, 128], F16, kind="ExternalInput").ap()
    dW2A = nc.dram_tensor("w2a", [128, 256], FP8, kind="ExternalInput").ap()
    dW2As = nc.dram_tensor("w2as", [128, 256], FP8, kind="ExternalInput").ap()
    dW2B = nc.dram_tensor("w2b", [128, 256], FP8, kind="ExternalInput").ap()
    dW2Bs = nc.dram_tensor("w2bs", [128, 256], FP8, kind="ExternalInput").ap()
    dW2C = nc.dram_tensor("w2c", [128, 256], FP8, kind="ExternalInput").ap()
    dW3D = nc.dram_tensor("w3d", [128, 768], FP8, kind="ExternalInput").ap()
    dW3D3 = nc.dram_tensor("w3d3", [128, 256], FP8, kind="ExternalInput").ap()
    dW3S = nc.dram_tensor("w3s", [128, 256], FP8, kind="ExternalInput").ap()
    dW4DA = nc.dram_tensor("w4da", [128, 768], FP8, kind="ExternalInput").ap()
    dW4D3A = nc.dram_tensor("w4d3a", [128, 256], FP8,
                            kind="ExternalInput").ap()
    dW4SA = nc.dram_tensor("w4sa", [128, 256], FP8, kind="ExternalInput").ap()
    dW4DB = nc.dram_tensor("w4db", [128, 384], FP8, kind="ExternalInput").ap()
    dW4D3B = nc.dram_tensor("w4d3b", [128, 128], FP8,
                            kind="ExternalInput").ap()
    dW4SB = nc.dram_tensor("w4sb", [128, 128], FP8, kind="ExternalInput").ap()
    dW5DA = nc.dram_tensor("w5da", [128, 256], FP8, kind="ExternalInput").ap()
    dW5DB = nc.dram_tensor("w5db", [128, 128], FP8, kind="ExternalInput").ap()
    dT1 = nc.dram_tensor("t1", [128, 1], F32, kind="ExternalInput").ap()
    dT2 = nc.dram_tensor("t2", [128, 1], F32, kind="ExternalInput").ap()
    dT3 = nc.dram_tensor("t3", [128, 1], F32, kind="ExternalInput").ap()
    dT4a = nc.dram_tensor("t4a", [128, 1], F32, kind="ExternalInput").ap()
    dT4b = nc.dram_tensor("t4b", [128, 1], F32, kind="ExternalInput").ap()
    dA5a = nc.dram_tensor("a5a", [128, 1], F32, kind="ExternalInput").ap()
    dA5b = nc.dram_tensor("a5b", [64, 1], F32, kind="ExternalInput").ap()
    dB5a = nc.dram_tensor("b5a", [128, 1], F32, kind="ExternalInput").ap()
    dB5b = nc.dram_tensor("b5b", [64, 1], F32, kind="ExternalInput").ap()
    dWTa = nc.dram_tensor("wta", [128, 12], F32, kind="ExternalInput").ap()
    dWTb = nc.dram_tensor("wtb", [65, 12], F32, kind="ExternalInput").ap()
    dY = nc.dram_tensor("y", [B, 12], F32, kind="ExternalOutput").ap()

    with tile.TileContext(nc) as tc:
        with tc.tile_pool(name="const", bufs=1) as cp, \
             tc.tile_pool(name="work", bufs=2) as wp, \
             tc.tile_pool(name="psum", bufs=5, space="PSUM") as pp:

            def ctile(name, shape, dtype):
                return cp.tile(shape, dtype, tag=name, name=name)

            # ---- persistent weight/param tiles ----
            cW1T = ctile("cW1T", [128, 128], F16)
            cW2A = ctile("cW2A", [128, 256], FP8)
            cW2As = ctile("cW2As", [128, 256], FP8)
            cW2B = ctile("cW2B", [128, 256], FP8)
            cW2Bs = ctile("cW2Bs", [128, 256], FP8)
            cW2C = ctile("cW2C", [128, 256], FP8)
            cW3D = ctile("cW3D", [128, 768], FP8)
            cW3D3 = ctile("cW3D3", [128, 256], FP8)
            cW3S = ctile("cW3S", [128, 256], FP8)
            cW4DA = ctile("cW4DA", [128, 768], FP8)
            cW4D3A = ctile("cW4D3A", [128, 256], FP8)
            cW4SA = ctile("cW4SA", [128, 256], FP8)
            cW4DB = ctile("cW4DB", [128, 384], FP8)
            cW4D3B = ctile("cW4D3B", [128, 128], FP8)
            cW4SB = ctile("cW4SB", [128, 128], FP8)
            cW5DA = ctile("cW5DA", [128, 256], FP8)
            cW5DB = ctile("cW5DB", [128, 128], FP8)
            cT1 = ctile("cT1", [128, 1], F32)
            cT2 = ctile("cT2", [128, 1], F32)
            cT3 = ctile("cT3", [128, 1], F32)
            cT4a = ctile("cT4a", [128, 1], F32)
            cT4b = ctile("cT4b", [128, 1], F32)
            cA5a = ctile("cA5a", [128, 1], F32)
            cA5b = ctile("cA5b", [64, 1], F32)
            cB5a = ctile("cB5a", [128, 1], F32)
            cB5b = ctile("cB5b", [64, 1], F32)
            cWTa = ctile("cWTa", [128, 12], F32)
            cWTb = ctile("cWTb", [65, 12], F32)

            for t_, d_ in [(cW1T, dW1T), (cW2A, dW2A), (cW2As, dW2As),
                           (cW2B, dW2B), (cW2Bs, dW2Bs), (cW2C, dW2C),
                           (cW3D, dW3D), (cW3D3, dW3D3), (cW3S, dW3S),
                           (cW4DA, dW4DA), (cW4D3A, dW4D3A), (cW4SA, dW4SA),
                           (cW4DB, dW4DB), (cW4D3B, dW4D3B), (cW4SB, dW4SB),
                           (cW5DA, dW5DA), (cW5DB, dW5DB), (cT1, dT1),
                           (cT2, dT2), (cT3, dT3), (cT4a, dT4a), (cT4b, dT4b),
                           (cA5a, dA5a), (cA5b, dA5b), (cB5a, dB5a),
                           (cB5b, dB5b), (cWTa, dWTa), (cWTb, dWTb)]:
                nc.sync.dma_start(t_[:], d_[:])

            # ---- persistent activation buffers ----
            IC = [ctile(f"IC{p}", [128, 4096], F16) for p in range(2)]
            NA1 = 8
            NAB = 3
            # A1 {0,1}: interior half (par-dependent) + +1-row copy on the
            # other half; +1-col block at BO1.
            A1 = [ctile(f"A1_{p}", [128, 2 * BO1], FP8) for p in range(NA1)]
            # A2 {-1,+1}: pitch 80; +1-col block at BO2
            A2 = [ctile(f"A2_{p}", [128, 2 * BO2], FP8) for p in range(NAB)]
            # A3 {0,1}: pitch 48; +1-col block at BO3
            A3 = [ctile(f"A3_{p}", [128, 2 * S3A], FP8) for p in range(NAB)]
            # A4 {0,1}: block0 = ch 0-127; block1 (offset NPOS) = ch 128-191
            # on partitions 0-63, zeros above
            A4 = [ctile(f"A4_{p}", [128, 2 * NPOS], FP8) for p in range(NAB)]
            MACCa = ctile("MACCa", [128, 2 * B], F32)
            MACCb = ctile("MACCb", [64, 2 * B], F32)
            Msum = ctile("Msum", [128, B], F32)
            MsumB = ctile("MsumB", [65, B], F32)

            # pad fill once; interiors are rewritten every image.
            for p in range(NA1):
                nc.gpsimd.memset(A1[p][:], 0.0)
            for p in range(NAB):
                nc.gpsimd.memset(A2[p][:], -1.0)
                nc.gpsimd.memset(A3[p][:], 0.0)
                nc.gpsimd.memset(A4[p][64:128, NPOS:2 * NPOS], 0.0)
            nc.vector.memset(MsumB[64:65, :], 1.0)

            w2a = cW2A[:].rearrange("p (j m) -> p j m", j=2)
            w2as = cW2As[:].rearrange("p (j m) -> p j m", j=2)
            w2b = cW2B[:].rearrange("p (j m) -> p j m", j=2)
            w2bs = cW2Bs[:].rearrange("p (j m) -> p j m", j=2)

            # ------ conv1 + binarize -> A1 (4 images per PE slot) ------
            def conv1_group(g):
                pq = g % 2
                nc.sync.dma_start(IC[pq][0:54, :], dX[2 * g])
                nc.sync.dma_start(IC[pq][64:118, :], dX[2 * g + 1])
                imgs = [4 * g + k for k in range(4)]
                bufs = [A1[im % NA1] for im in imgs]
                for r in range(8):
                    pss = []
                    for h in range(2):
                        o = h * 64
                        ps = pp.tile([128, 512], F32, tag="mm",
                                     name=f"ps_c1_{g}_{r}_{h}")
                        nc.tensor.matmul(
                            ps[:], cW1T[o:o + 54, :],
                            IC[pq][o:o + 54, r * 512:(r + 1) * 512],
                            start=True, stop=True)
                        pss.append(ps)
                    for h in range(2):
                        for par in range(2):
                            a1t = bufs[2 * h + par]
                            lo = par * 64
                            a1vv = a1t[lo:lo + 64, 0:S2A].rearrange(
                                "p (a b) -> p a b", b=PA)
                            nc.vector.tensor_scalar(
                                a1vv[:, r * 8 + 1:r * 8 + 9, 1:65],
                                pss[h][par * 64:par * 64 + 64, :].rearrange(
                                    "q (a b) -> q a b", b=64),
                                cT1[par * 64:par * 64 + 64], None, OP.is_gt)
                for k in range(4):
                    a1t = bufs[k]
                    par = k % 2
                    lo, hi = par * 64, 64 - par * 64
                    # +1-row copy on the other partition half
                    nc.sync.dma_start(a1t[hi:hi + 64, 0:S2A - PA],
                                      a1t[lo:lo + 64, PA:S2A])
                    # +1-col block (both halves)
                    nc.sync.dma_start(a1t[:, BO1:BO1 + S1A - 1],
                                      a1t[:, 1:S1A])

            # ---------------- layer bodies ----------------
            def l2_block(i):
                p = i % NAB
                par = i % 2
                a1f = A1[i % NA1][:, :]
                wa = w2a if par == 0 else w2as
                wb = w2b if par == 0 else w2bs
                lo = par * 64
                hi = 64 - lo
                a2vv = A2[p][:, 0:S2A].rearrange("p (a b) -> p a b", b=PA)
                for cpair in range(4):
                    c0, c1 = 2 * cpair, 2 * cpair + 1
                    pss = {}
                    for c in (c0, c1):
                        q0 = (c * 8) * PA
                        ps = pp.tile([128, 512], F32, tag="mm",
                                     name=f"ps_l2_{i}_{c}")
                        psv = ps[:].rearrange("q (a b) -> q a b", b=64)
                        pss[c] = (psv, q0)
                        # MM1: (0,0),(1,0) | +1col: (0,1),(1,1)
                        nc.tensor.matmul(
                            psv, wa,
                            _ap(a1f, q0, [[BO1, 2], [PA, 8], [1, 64]]),
                            start=True, stop=False, perf_mode=DR)
                        # MM2: (0,2),(1,2) | +2rows: (2,2),(zero)
                        nc.tensor.matmul(
                            psv, wb,
                            _ap(a1f, q0 + 2, [[160, 2], [PA, 8], [1, 64]]),
                            start=False, stop=False, perf_mode=DR)
                    # leftover {(2,0),(2,1)}: K=64 units, row-tiled pair
                    nc.tensor.matmul(
                        pss[c0][0], cW2C[lo:lo + 64, :].rearrange(
                            "p (j m) -> p j m", j=2),
                        _ap(a1f[lo:lo + 64, :], pss[c0][1] + 160,
                            [[BO1, 2], [PA, 8], [1, 64]]),
                        start=False, stop=True, perf_mode=DR)
                    nc.tensor.matmul(
                        pss[c1][0], cW2C[hi:hi + 64, :].rearrange(
                            "p (j m) -> p j m", j=2),
                        _ap(a1f[hi:hi + 64, :], pss[c1][1] + PA,
                            [[BO1, 2], [PA, 8], [1, 64]]),
                        start=False, stop=True, perf_mode=DR)
                    for c in (c0, c1):
                        y0 = c * 8
                        nc.scalar.activation(
                            a2vv[:, y0 + 1:y0 + 9, 1:65], pss[c][0],
                            ACT.Sign, bias=cT2[:], scale=1.0)
                    # per-pair +1-col block copy (rows 16*cpair+1 .. +16)
                    ql = (16 * cpair + 1) * PA
                    qh = (16 * cpair + 16) * PA + 66
                    nc.sync.dma_start(
                        A2[p][:, BO2 + ql - 1:BO2 + qh - 1],
                        A2[p][:, ql:qh])

            def l3_block(i):
                p = i % NAB
                a2f = A2[p][:, :]
                a3v = A3[p][:, 0:S3A].rearrange("p (a b) -> p a b", b=P3A)
                for r in range(2):
                    y0 = r * 16
                    base = (2 * y0) * PA
                    ps = pp.tile([128, 512], F32, tag="mm",
                                 name=f"ps_l3_{i}_{r}")
                    psv = ps[:].rearrange("q (a b) -> q a b", b=32)
                    for kx in range(3):
                        nc.tensor.matmul(
                            psv,
                            cW3D[:, kx * 256:(kx + 1) * 256]
                            .rearrange("p (j m) -> p j m", j=2),
                            _ap(a2f, base + kx,
                                [[PA, 2], [2 * PA, 16], [2, 32]]),
                            start=(kx == 0), stop=False, perf_mode=DR)
                    nc.tensor.matmul(
                        psv, cW3D3[:].rearrange("p (j m) -> p j m", j=2),
                        _ap(a2f, base + 2 * PA,
                            [[BO2, 2], [2 * PA, 16], [2, 32]]),
                        start=False, stop=False, perf_mode=DR)
                    nc.tensor.matmul(
                        psv, cW3S[:].rearrange("p (j m) -> p j m", j=2),
                        _ap(a2f, base + 2 * PA + 2,
                            [[16, 2], [2 * PA, 16], [2, 32]]),
                        start=False, stop=True, perf_mode=DR)
                    # binarize {0,1} on DVE
                    nc.vector.tensor_scalar(
                        a3v[:, y0 + 1:y0 + 17, 1:33], psv,
                        cT3[:], None, OP.is_gt)
                # +1-col block (one DMA per image)
                nc.sync.dma_start(
                    A3[p][:, BO3:BO3 + S3A - 1], A3[p][:, 1:S3A])

            def l4_block(i):
                p = i % NAB
                a3f = A3[p][:, :]
                a4av = A4[p][:, 0:NPOS].rearrange("p (a b) -> p a b", b=32)
                a4bv = A4[p][0:64, NPOS:2 * NPOS].rearrange(
                    "p (a b) -> p a b", b=32)
                for ci in range(2):
                    y0 = ci * 16
                    q0 = y0 * P3A
                    psa = pp.tile([128, 512], F32, tag="mm",
                                  name=f"ps_l4a_{i}_{ci}")
                    psb = pp.tile([64, 512], F32, tag="mb2", bufs=2,
                                  name=f"ps_l4b_{i}_{ci}")
                    for mb in range(2):
                        psx = psa if mb == 0 else psb
                        wd = cW4DA if mb == 0 else cW4DB
                        wd3 = cW4D3A if mb == 0 else cW4D3B
                        ws = cW4SA if mb == 0 else cW4SB
                        mw = 128 if mb == 0 else 64
                        for kx in range(3):
                            nc.tensor.matmul(
                                psx[:],
                                wd[:, kx * 2 * mw:(kx + 1) * 2 * mw]
                                .rearrange("p (j m) -> p j m", j=2),
                                _ap(a3f, q0 + kx,
                                    [[P3A, 2], [P3A, 16], [1, 32]]),
                                start=(kx == 0), stop=False, perf_mode=DR)
                        nc.tensor.matmul(
                            psx[:],
                            wd3[:, 0:2 * mw].rearrange(
                                "p (j m) -> p j m", j=2),
                            _ap(a3f, q0 + 2 * P3A,
                                [[BO3, 2], [P3A, 16], [1, 32]]),
                            start=False, stop=False, perf_mode=DR)
                        nc.tensor.matmul(
                            psx[:],
                            ws[:, 0:2 * mw].rearrange("p (j m) -> p j m", j=2),
                            _ap(a3f, q0 + 2 * P3A + 2,
                                [[16, 2], [P3A, 16], [1, 32]]),
                            start=False, stop=True, perf_mode=DR)
                    nc.vector.tensor_scalar(
                        a4av[:, y0:y0 + 16, 0:32],
                        psa[:].rearrange("q (a b) -> q a b", b=32),
                        cT4a[:], None, OP.is_gt)
                    nc.vector.tensor_scalar(
                        a4bv[:, y0:y0 + 16, 0:32],
                        psb[:].rearrange("q (a b) -> q a b", b=32),
                        cT4b[0:64], None, OP.is_gt)

            def l5_block(i):
                p = i % NAB
                a4f = A4[p][:, :]
                for c in range(2):
                    psa = pp.tile([128, 512], F32, tag="mm",
                                  name=f"ps_l5a_{i}_{c}")
                    psb = pp.tile([64, 512], F32, tag="mb2", bufs=2,
                                  name=f"ps_l5b_{i}_{c}")
                    rhs = _ap(a4f, c * 512, [[NPOS, 2], [1, 512]])
                    nc.tensor.matmul(
                        psa[:], cW5DA[:].rearrange("p (j m) -> p j m", j=2),
                        rhs, start=True, stop=True, perf_mode=DR)
                    nc.tensor.matmul(
                        psb[:], cW5DB[:].rearrange("p (j m) -> p j m", j=2),
                        rhs, start=True, stop=True, perf_mode=DR)
                    scra = wp.tile([128, 512], F32, tag="scr_a",
                                   name=f"scra_{i}_{c}")
                    scrb = wp.tile([64, 512], F32, tag="scr_b",
                                   name=f"scrb_{i}_{c}")
                    nc.scalar.activation(
                        scra[:], psa[:], ACT.Relu, bias=cB5a[:], scale=cA5a[:],
                        accum_out=MACCa[:, 2 * i + c: 2 * i + c + 1])
                    nc.scalar.activation(
                        scrb[:], psb[:], ACT.Relu, bias=cB5b[:], scale=cA5b[:],
                        accum_out=MACCb[:, 2 * i + c: 2 * i + c + 1])

            # ---------------- main pipeline ----------------
            START = {1: 1, 5: 2, 9: 3}
            for _rep in range(reps):
                conv1_group(0)
                for i in range(B):
                    if i in START:
                        conv1_group(START[i])
                    l2_block(i)
                    l3_block(i)
                    l4_block(i)
                    l5_block(i)

            # ---------------- GAP/FC/softmax tail ----------------
            nc.vector.tensor_reduce(
                Msum[:, 0:B], MACCa[:].rearrange("p (i c) -> p i c", c=2),
                axis=AX.X, op=OP.add)
            nc.vector.tensor_reduce(
                MsumB[0:64, 0:B], MACCb[:].rearrange("p (i c) -> p i c", c=2),
                axis=AX.X, op=OP.add)

            psf = pp.tile([64, 512], F32, tag="mb2", bufs=2, name="ps_fc")
            nc.tensor.matmul(psf[0:16, 0:12], Msum[:, 0:B], cWTa[:],
                             start=True, stop=False)
            nc.tensor.matmul(psf[0:16, 0:12], MsumB[:, 0:B], cWTb[:],
                             start=False, stop=True)

            negmax = cp.tile([16, 1], F32, tag="negmax", name="negmax")
            esum = cp.tile([16, 1], F32, tag="esum", name="esum")
            rsum = cp.tile([16, 1], F32, tag="rsum", name="rsum")
            etile = cp.tile([16, 12], F32, tag="etile", name="etile")
            yout = cp.tile([16, 12], F32, tag="yout", name="yout")

            nc.vector.tensor_reduce(negmax[:], psf[0:16, 0:12], axis=AX.X,
                                    op=OP.max, negate=True)
            nc.scalar.activation(etile[:], psf[0:16, 0:12], ACT.Exp,
                                 bias=negmax[:], scale=1.0, accum_out=esum[:])
            nc.vector.reciprocal(rsum[:], esum[:])
            nc.vector.tensor_scalar(yout[:], etile[:], rsum[:], None, OP.mult)
            nc.sync.dma_start(dY[:], yout[:])

    nc.compile()
    _CACHE[key] = nc
    return _CACHE


def _host_prep(inputs):
    """Fold BN into thresholds/affines; sign-binarize weights; build per-core
    input maps."""
    f32 = np.float32
    fp8 = mybir.dt.np(FP8)

    x = np.asarray(inputs["x"], f32)

    def inv(l):
        return (np.asarray(inputs[f"bn{l}_g"], f32)
                / np.sqrt(np.asarray(inputs[f"bn{l}_v"], f32)
                          + np.float32(EPS)))

    invs = {l: inv(l) for l in (1, 2, 3, 4, 5)}
    for l in (1, 2, 3, 4):
        assert (invs[l] > 0).all(), f"bn{l} scale not positive"

    def thr(l):
        return (np.asarray(inputs[f"bn{l}_m"], f32)
                - np.asarray(inputs[f"bn{l}_b"], f32) / invs[l])

    sw2 = np.sign(np.asarray(inputs["w2"], f32))       # [128,64,3,3]
    sw3 = np.sign(np.asarray(inputs["w3"], f32))       # [128,128,3,3]
    sw4 = np.sign(np.asarray(inputs["w4"], f32))       # [192,128,3,3]
    sw5 = np.sign(np.asarray(inputs["w5"], f32))       # [192,192,1,1]

    t1v = thr(1) - np.asarray(inputs["conv1_b"], f32)
    t1 = np.concatenate([t1v, t1v]).reshape(128, 1)
    # A1 {0,1}; A2 {-1,+1} via ScalarE Sign (bias = -thr2)
    t2 = (-thr(2)).reshape(128, 1)
    # L3 consumes pm1 A2, emits {0,1} A3 via is_gt with folded threshold
    sw3sum = sw3.sum(axis=(1, 2, 3))
    t3 = (2.0 * thr(3) - sw3sum).reshape(128, 1)
    # L4 consumes {0,1} A3 -> plain thresholds
    t4 = thr(4)
    a5 = invs[5]
    b5 = (np.asarray(inputs["bn5_b"], f32)
          - np.asarray(inputs["bn5_m"], f32) * invs[5])

    # conv1 weights: block-diag [54, 128] replicated at rows 0/64.
    w1 = np.asarray(inputs["conv1_w"], f32)           # [64,1,3,3]
    w1t = np.ascontiguousarray(w1[:, 0].reshape(64, 9).T)  # [9, 64]
    w1hi = w1t.astype(np.float16)
    w1lo = (w1t - w1hi.astype(f32)).astype(np.float16)
    w1t27 = np.concatenate(
        [w1hi, w1lo, (w1hi.astype(f32) / 64.0).astype(np.float16)], axis=0)
    w1t4 = np.zeros((128, 128), np.float16)
    w1t4[0:27, 0:64] = w1t27
    w1t4[27:54, 64:128] = w1t27
    w1t4[64:91, 0:64] = w1t27
    w1t4[91:118, 64:128] = w1t27

    # L2 packs (normal layout: partitions 0-63 = unshifted, 64-127 = +1 row)
    w2a_ = np.zeros((128, 2, 128), f32)
    w2a_[0:64, 0] = sw2[:, :, 0, 0].T
    w2a_[64:128, 0] = sw2[:, :, 1, 0].T
    w2a_[0:64, 1] = sw2[:, :, 0, 1].T
    w2a_[64:128, 1] = sw2[:, :, 1, 1].T
    w2b_ = np.zeros((128, 2, 128), f32)
    w2b_[0:64, 0] = sw2[:, :, 0, 2].T
    w2b_[64:128, 0] = sw2[:, :, 1, 2].T
    w2b_[0:64, 1] = sw2[:, :, 2, 2].T
    w2c_ = np.zeros((128, 2, 128), f32)
    w2c_[0:64, 0] = sw2[:, :, 2, 0].T
    w2c_[0:64, 1] = sw2[:, :, 2, 1].T
    w2c_[64:128, 0] = sw2[:, :, 2, 0].T
    w2c_[64:128, 1] = sw2[:, :, 2, 1].T
    w2as_ = np.concatenate([w2a_[64:128], w2a_[0:64]], axis=0)
    w2bs_ = np.concatenate([w2b_[64:128], w2b_[0:64]], axis=0)

    # L3 packs: w3d[kx]: j = ky in {0,1}; w3d3: j0=(2,0), j1=(2,1); w3s solo
    w3d = np.zeros((128, 3, 2, 128), f32)
    for kx in range(3):
        for j in range(2):
            w3d[:, kx, j] = sw3[:, :, j, kx].T
    w3d3 = np.zeros((128, 2, 128), f32)
    w3d3[:, 0] = sw3[:, :, 2, 0].T
    w3d3[:, 1] = sw3[:, :, 2, 1].T
    w3s = np.concatenate([sw3[:, :, 2, 2].T, np.zeros((128, 128), f32)],
                         axis=1)

    # L4 packs (baseline layout)
    w4da = np.zeros((128, 3, 2, 128), f32)
    w4db = np.zeros((128, 3, 2, 64), f32)
    for kx in range(3):
        for j in range(2):
            w4da[:, kx, j] = sw4[:128, :, j, kx].T
            w4db[:, kx, j] = sw4[128:, :, j, kx].T
    w4d3a = np.zeros((128, 2, 128), f32)
    w4d3b = np.zeros((128, 2, 64), f32)
    for j in range(2):
        w4d3a[:, j] = sw4[:128, :, 2, j].T
        w4d3b[:, j] = sw4[128:, :, 2, j].T
    w4sa = np.concatenate([sw4[:128, :, 2, 2].T, np.zeros((128, 128), f32)],
                          axis=1)
    w4sb = np.concatenate([sw4[128:, :, 2, 2].T, np.zeros((128, 64), f32)],
                          axis=1)

    # L5 packs (baseline layout)
    w5 = sw5[:, :, 0, 0]
    w5da = np.zeros((128, 2, 128), f32)
    w5da[:, 0] = w5[:128, :128].T
    w5da[0:64, 1] = w5[:128, 128:].T
    w5db = np.zeros((128, 2, 64), f32)
    w5db[:, 0] = w5[128:, :128].T
    w5db[0:64, 1] = w5[128:, 128:].T

    fc_w = np.asarray(inputs["fc_w"], f32)
    c6w = np.asarray(inputs["conv6_w"], f32)[:, :, 0, 0]
    Wp = (fc_w @ c6w) / np.float32(NPOS)
    cvec = fc_w @ np.asarray(inputs["conv6_b"], f32) + np.asarray(
        inputs["fc_b"], f32)
    wta = np.ascontiguousarray(Wp[:, :128].T)
    wtb = np.zeros((65, 12), f32)
    wtb[:64] = Wp[:, 128:].T
    wtb[64] = cvec

    shared = {
        "w1t": w1t4,
        "w2a": w2a_.reshape(128, 256).astype(fp8),
        "w2as": w2as_.reshape(128, 256).astype(fp8),
        "w2b": w2b_.reshape(128, 256).astype(fp8),
        "w2bs": w2bs_.reshape(128, 256).astype(fp8),
        "w2c": w2c_.reshape(128, 256).astype(fp8),
        "w3d": w3d.reshape(128, 768).astype(fp8),
        "w3d3": w3d3.reshape(128, 256).astype(fp8),
        "w3s": w3s.astype(fp8),
        "w4da": w4da.reshape(128, 768).astype(fp8),
        "w4d3a": w4d3a.reshape(128, 256).astype(fp8),
        "w4sa": w4sa.astype(fp8),
        "w4db": w4db.reshape(128, 384).astype(fp8),
        "w4d3b": w4d3b.reshape(128, 128).astype(fp8),
        "w4sb": w4sb.astype(fp8),
        "w5da": w5da.reshape(128, 256).astype(fp8),
        "w5db": w5db.reshape(128, 128).astype(fp8),
        "t1": t1.astype(f32), "t2": t2.astype(f32), "t3": t3.astype(f32),
        "t4a": t4[:128].reshape(128, 1).astype(f32),
        "t4b": np.concatenate([t4[128:], t4[128:]]).reshape(128, 1)
        .astype(f32),
        "a5a": a5[:128].reshape(128, 1).astype(f32),
        "a5b": a5[128:].reshape(64, 1).astype(f32),
        "b5a": b5[:128].reshape(128, 1).astype(f32),
        "b5b": b5[128:].reshape(64, 1).astype(f32),
        "wta": wta.astype(f32), "wtb": wtb.astype(f32),
    }
    # host im2col: cols[b, 3*ky+kx, y*64+x] = xpad[b, 2y+ky, 2x+kx]
    xpad = np.pad(x[:, 0], ((0, 0), (1, 1), (1, 1)))
    cols = np.stack([xpad[:, ky:ky + 127:2, kx:kx + 127:2]
                     for ky in range(3) for kx in range(3)],
                    axis=1).reshape(x.shape[0], 9, 4096)
    chi = cols.astype(np.float16)
    clo = ((cols - chi.astype(f32)) * 64.0).astype(np.float16)
    cols27 = np.concatenate([chi, chi, clo], axis=1)    # [128, 27, 4096]
    cols54 = np.ascontiguousarray(
        cols27.reshape(x.shape[0] // 2, 54, 4096))      # image pairs
    in_maps = []
    for c in range(N_CORES):
        m = dict(shared)
        m["x"] = np.ascontiguousarray(cols54[c * (B // 2):(c + 1) * (B // 2)])
        in_maps.append(m)
    return in_maps


def kernel(**inputs):
    cache = _build()
    in_maps = _host_prep(inputs)
    res = run_bass_kernel_spmd(cache["nc1"], in_maps,
                               core_ids=list(range(N_CORES)))
    _CACHE["last_results"] = res
    return np.concatenate([res.results[c]["y"] for c in range(N_CORES)],
                          axis=0)


# ---------------------------------------------------------------------------
# numpy golden model of the device algorithm (validates packing w/o HW)
# ---------------------------------------------------------------------------
def golden(inputs):
    f32 = np.float32
    in_maps = _host_prep(inputs)
    outs = []
    for m in in_maps:
        cols = np.asarray(m["x"], f32).reshape(B, 27, 4096)
        t1 = m["t1"][:64, 0]
        w1t = np.asarray(m["w1t"][0:27, 0:64], f32)
        c1 = np.einsum("btn,tc->bcn", cols, w1t).reshape(-1, 64, 64, 64)
        a1 = (c1 > t1[None, :, None, None]).astype(f32)

        def bconv(a, wt, stride, pad_val=0.0):
            Bn, C, H, W = a.shape
            ap = np.pad(a, ((0, 0), (0, 0), (1, 1), (1, 1)),
                        constant_values=pad_val)
            Ho, Wo = H // stride, W // stride
            out = np.zeros((Bn, wt.shape[2], Ho, Wo), f32)
            for t in range(9):
                ky, kx = t // 3, t % 3
                sl = ap[:, :, ky:ky + H:stride, kx:kx + W:stride][
                    :, :, :Ho, :Wo]
                out += np.einsum("bcyx,cd->bdyx", sl, wt[:, t])
            return out

        # reconstruct w2 [ci, t, co] from packs (normal layout)
        w2a_ = np.asarray(m["w2a"], f32).reshape(128, 2, 128)
        w2b_ = np.asarray(m["w2b"], f32).reshape(128, 2, 128)
        w2c_ = np.asarray(m["w2c"], f32).reshape(128, 2, 128)
        w2 = np.zeros((64, 9, 128), f32)
        w2[:, 0] = w2a_[0:64, 0]     # (0,0)
        w2[:, 3] = w2a_[64:128, 0]   # (1,0)
        w2[:, 1] = w2a_[0:64, 1]     # (0,1)
        w2[:, 4] = w2a_[64:128, 1]   # (1,1)
        w2[:, 2] = w2b_[0:64, 0]     # (0,2)
        w2[:, 5] = w2b_[64:128, 0]   # (1,2)
        w2[:, 8] = w2b_[0:64, 1]     # (2,2)
        w2[:, 6] = w2c_[0:64, 0]     # (2,0)
        w2[:, 7] = w2c_[0:64, 1]     # (2,1)
        c2 = bconv(a1, w2, 1)
        a2 = np.sign(c2 + m["t2"].reshape(1, 128, 1, 1)).astype(f32)

        w3dg = np.asarray(m["w3d"], f32).reshape(128, 3, 2, 128)
        w3d3g = np.asarray(m["w3d3"], f32).reshape(128, 2, 128)
        w3 = np.zeros((128, 9, 128), f32)
        for kx in range(3):
            for j in range(2):
                w3[:, 3 * j + kx] = w3dg[:, kx, j]
        w3[:, 6] = w3d3g[:, 0]
        w3[:, 7] = w3d3g[:, 1]
        w3[:, 8] = np.asarray(m["w3s"], f32)[:, :128]
        c3 = bconv(a2, w3, 2, pad_val=-1.0)
        a3 = (c3 > m["t3"].reshape(1, 128, 1, 1)).astype(f32)

        w4da = np.asarray(m["w4da"], f32).reshape(128, 3, 2, 128)
        w4db = np.asarray(m["w4db"], f32).reshape(128, 3, 2, 64)
        w4d3a = np.asarray(m["w4d3a"], f32).reshape(128, 2, 128)
        w4d3b = np.asarray(m["w4d3b"], f32).reshape(128, 2, 64)
        w4 = np.zeros((128, 9, 192), f32)
        for kx in range(3):
            for j in range(2):
                w4[:, 3 * j + kx, :128] = w4da[:, kx, j]
                w4[:, 3 * j + kx, 128:] = w4db[:, kx, j]
        for j in range(2):
            w4[:, 6 + j, :128] = w4d3a[:, j]
            w4[:, 6 + j, 128:] = w4d3b[:, j]
        w4[:, 8, :128] = np.asarray(m["w4sa"], f32)[:, :128]
        w4[:, 8, 128:] = np.asarray(m["w4sb"], f32)[:, :64]
        c4 = bconv(a3, w4, 1, pad_val=0.0)
        a4 = np.concatenate([
            (c4[:, :128] > m["t4a"].reshape(1, 128, 1, 1)).astype(f32),
            (c4[:, 128:] > m["t4b"][:64].reshape(1, 64, 1, 1)).astype(f32)],
            axis=1)

        w5dag = np.asarray(m["w5da"], f32).reshape(128, 2, 128)
        w5dbg = np.asarray(m["w5db"], f32).reshape(128, 2, 64)
        w5 = np.zeros((192, 192), f32)
        w5[:128, :128] = w5dag[:, 0]
        w5[128:, :128] = w5dag[0:64, 1]
        w5[:128, 128:] = w5dbg[:, 0]
        w5[128:, 128:] = w5dbg[0:64, 1]
        c5 = np.einsum("bcyx,cd->bdyx", a4, w5)
        a5v = np.concatenate([m["a5a"], m["a5b"]], axis=0).reshape(
            1, 192, 1, 1)
        b5v = np.concatenate([m["b5a"], m["b5b"]], axis=0).reshape(
            1, 192, 1, 1)
        h5 = np.maximum(a5v * c5 + b5v, 0.0)
        sums = h5.sum(axis=(2, 3))
        WT = np.concatenate([m["wta"], m["wtb"][:64]], axis=0)
        logits = sums @ WT + m["wtb"][64][None, :]
        z = logits - logits.max(axis=1, keepdims=True)
        e = np.exp(z)
        outs.append(e / e.sum(axis=1, keepdims=True))
    return np.concatenate(outs, axis=0)


# revision 4
# speedup vs baseline: 1.0483x; 1.0483x over previous
"""Trainium2 Bass kernel for BinarizedInputNetwork — v2.

Contract: kernel(**inputs) takes the FULL unsharded inputs (batch 128) and
returns the FULL [128, 12] float32 softmax output. Internally shards the
batch across 8 NeuronCores (16 images each), runs one SPMD Bass program.

v2 changes vs baseline:
  - conv1 as block-diagonal K=54 fp16 matmuls (2 images per MM: the pair's
    im2col on partitions 0-26/27-53, couts at psum 0-63/64-127) x row-tiled
    pairs at partition offsets {0,64} => 4 images per PE slot. Odd images
    keep their A1 partition halves swapped (psum-half alignment), with
    partition-swapped L2 weight packs.
  - A1 on a 16-aligned pitch (80) so DoubleRow j-steps can express row
    shifts directly; one +1-col shifted block. L2 repacked: MM1 covers
    (0,0),(1,0),(0,1),(1,1); MM2 covers (0,2),(1,2),(2,2) via j-step
    +2 rows; leftover {(2,0),(2,1)} is a K=64 unit row-tile-paired across
    chunk pairs (two concurrent half-K matmuls) -> 2.5 slots/chunk.
  - A3 switched to {0,1} coding via DVE is_gt (moves L3 binarize off the
    Activation engine); thresholds folded accordingly.
  - Shifted-copy DMAs merged (A1 copy2+block, A3 block: one DMA each).
"""

import sys

sys.path.insert(0, "/opt/trn_rl_repo")

import numpy as np

import concourse.ap as apm
import concourse.bass as bass
import concourse.mybir as mybir
import concourse.bacc as bacc
import concourse.tile as tile
from concourse.bass_utils import run_bass_kernel_spmd

F32 = mybir.dt.float32
F16 = mybir.dt.float16
FP8 = mybir.dt.float8e4
AX = mybir.AxisListType
OP = mybir.AluOpType
ACT = mybir.ActivationFunctionType
DR = mybir.MatmulPerfMode.DoubleRow

N_CORES = 8
B = 16  # images per core

EPS = 1e-5

# geometry
PA = 80                  # A1/A2 row pitch (16-aligned)
S1A = PA * 67            # A1 area incl guard row (5360)
BO1 = 5376               # A1 +1-col block offset (16-aligned)
S2A = PA * 66            # A2 interior area (5280)
BO2 = 5376               # A2 +1-col block offset
P3A = 48                 # A3 row pitch (34 rows x 48)
S3A = 34 * P3A           # 1632
BO3 = 1632               # A3 +1-col block offset (16-aligned)
NPOS = 1024              # 32*32 valid positions for L5/GAP

_CACHE = {}


def _ap(base2d, off, dims):
    """Custom AP over an SBUF tile slice: base partition dim + free dims
    (supports overlapping patterns rearrange can't express)."""
    return apm.AP(tensor=base2d.tensor, offset=base2d.offset + off,
                  ap=[list(base2d.ap[0])] + [list(d) for d in dims])


def _build(reps=1):
    key = f"nc{reps}"
    if key in _CACHE:
        return _CACHE

    nc = bacc.Bacc("TRN2", target_bir_lowering=False, debug=False,
                   num_devices=N_CORES)

    # ---- DRAM I/O ----
    # host-side im2col, fp16 split-precision, packed in image PAIRS:
    # [8, 54, 4096]: rows 0-26 = even img taps (hi,hi,lo*64), 27-53 = odd.
    dX = nc.dram_tensor("x", [B // 2, 54, 4096], F16,
                        kind="ExternalInput").ap()
    dW1T = nc.dram_tensor("w1t", [128, 128], F16, kind="ExternalInput").ap()
    dW2A = nc.dram_tensor("w2a", [128, 256], FP8, kind="ExternalInput").ap()
    dW2As = nc.dram_tensor("w2as", [128, 256], FP8, kind="ExternalInput").ap()
    dW2B = nc.dram_tensor("w2b", [128, 256], FP8, kind="ExternalInput").ap()
    dW2Bs = nc.dram_tensor("w2bs", [128, 256], FP8, kind="ExternalInput").ap()
    dW2C = nc.dram_tensor("w2c", [128, 256], FP8, kind="ExternalInput").ap()
    dW3D = nc.dram_tensor("w3d", [128, 768], FP8, kind="ExternalInput").ap()
    dW3D3 = nc.dram_tensor("w3d3", [128, 256], FP8, kind="ExternalInput").ap()
    dW3S = nc.dram_tensor("w3s", [128, 256], FP8, kind="ExternalInput").ap()
    dW4DA = nc.dram_tensor("w4da", [128, 768], FP8, kind="ExternalInput").ap()
    dW4D3A = nc.dram_tensor("w4d3a", [128, 256], FP8,
                            kind="ExternalInput").ap()
    dW4SA = nc.dram_tensor("w4sa", [128, 256], FP8, kind="ExternalInput").ap()
    dW4DB = nc.dram_tensor("w4db", [128, 384], FP8, kind="ExternalInput").ap()
    dW4D3B = nc.dram_tensor("w4d3b", [128, 128], FP8,
                            kind="ExternalInput").ap()
    dW4SB = nc.dram_tensor("w4sb", [128, 128], FP8, kind="ExternalInput").ap()
    dW5DA = nc.dram_tensor("w5da", [128, 256], FP8, kind="ExternalInput").ap()
    dW5DB = nc.dram_tensor("w5db", [128, 128], FP8, kind="ExternalInput").ap()
    dT1 = nc.dram_tensor("t1", [128, 1], F32, kind="ExternalInput").ap()
    dT2 = nc.dram_tensor("t2", [128, 1], F32, kind="ExternalInput").ap()
    dT3 = nc.dram_tensor("t3", [128, 1], F32, kind="ExternalInput").ap()
    dT4a = nc.dram_tensor("t4a", [128, 1], F32, kind="ExternalInput").ap()
    dT4b = nc.dram_tensor("t4b", [128, 1], F32, kind="ExternalInput").ap()
    dA5a = nc.dram_tensor("a5a", [128, 1], F32, kind="ExternalInput").ap()
    dA5b = nc.dram_tensor("a5b", [64, 1], F32, kind="ExternalInput").ap()
    dB5a = nc.dram_tensor("b5a", [128, 1], F32, kind="ExternalInput").ap()
    dB5b = nc.dram_tensor("b5b", [64, 1], F32, kind="ExternalInput").ap()
    dWTa = nc.dram_tensor("wta", [128, 12], F32, kind="ExternalInput").ap()
    dWTb = nc.dram_tensor("wtb", [65, 12], F32, kind="ExternalInput").ap()
    dY = nc.dram_tensor("y", [B, 12], F32, kind="ExternalOutput").ap()

    with tile.TileContext(nc) as tc:
        with tc.tile_pool(name="const", bufs=1) as cp, \
             tc.tile_pool(name="work", bufs=2) as wp, \
             tc.tile_pool(name="psum", bufs=6, space="PSUM") as pp:

            def ctile(name, shape, dtype):
                return cp.tile(shape, dtype, tag=name, name=name)

            # ---- persistent weight/param tiles ----
            cW1T = ctile("cW1T", [128, 128], F16)
            cW2A = ctile("cW2A", [128, 256], FP8)
            cW2As = ctile("cW2As", [128, 256], FP8)
            cW2B = ctile("cW2B", [128, 256], FP8)
            cW2Bs = ctile("cW2Bs", [128, 256], FP8)
            cW2C = ctile("cW2C", [128, 256], FP8)
            cW3D = ctile("cW3D", [128, 768], FP8)
            cW3D3 = ctile("cW3D3", [128, 256], FP8)
            cW3S = ctile("cW3S", [128, 256], FP8)
            cW4DA = ctile("cW4DA", [128, 768], FP8)
            cW4D3A = ctile("cW4D3A", [128, 256], FP8)
            cW4SA = ctile("cW4SA", [128, 256], FP8)
            cW4DB = ctile("cW4DB", [128, 384], FP8)
            cW4D3B = ctile("cW4D3B", [128, 128], FP8)
            cW4SB = ctile("cW4SB", [128, 128], FP8)
            cW5DA = ctile("cW5DA", [128, 256], FP8)
            cW5DB = ctile("cW5DB", [128, 128], FP8)
            cT1 = ctile("cT1", [128, 1], F32)
            cT2 = ctile("cT2", [128, 1], F32)
            cT3 = ctile("cT3", [128, 1], F32)
            cT4a = ctile("cT4a", [128, 1], F32)
            cT4b = ctile("cT4b", [128, 1], F32)
            cA5a = ctile("cA5a", [128, 1], F32)
            cA5b = ctile("cA5b", [64, 1], F32)
            cB5a = ctile("cB5a", [128, 1], F32)
            cB5b = ctile("cB5b", [64, 1], F32)
            cWTa = ctile("cWTa", [128, 12], F32)
            cWTb = ctile("cWTb", [65, 12], F32)
            cZ = ctile("cZ", [128, 1], F32)
            nc.vector.memset(cZ[:], 0.0)

            for t_, d_ in [(cW1T, dW1T), (cW2A, dW2A), (cW2As, dW2As),
                           (cW2B, dW2B), (cW2Bs, dW2Bs), (cW2C, dW2C),
                           (cW3D, dW3D), (cW3D3, dW3D3), (cW3S, dW3S),
                           (cW4DA, dW4DA), (cW4D3A, dW4D3A), (cW4SA, dW4SA),
                           (cW4DB, dW4DB), (cW4D3B, dW4D3B), (cW4SB, dW4SB),
                           (cW5DA, dW5DA), (cW5DB, dW5DB), (cT1, dT1),
                           (cT2, dT2), (cT3, dT3), (cT4a, dT4a), (cT4b, dT4b),
                           (cA5a, dA5a), (cA5b, dA5b), (cB5a, dB5a),
                           (cB5b, dB5b), (cWTa, dWTa), (cWTb, dWTb)]:
                nc.sync.dma_start(t_[:], d_[:])

            # ---- persistent activation buffers ----
            IC = [ctile(f"IC{p}", [128, 4096], F16) for p in range(2)]
            NA1 = 8
            NAB = 3
            # A1 {0,1}: interior half (par-dependent) + +1-row copy on the
            # other half; +1-col block at BO1.
            A1 = [ctile(f"A1_{p}", [128, 2 * BO1], FP8) for p in range(NA1)]
            # A2 {-1,+1}: pitch 80; +1-col block at BO2
            A2 = [ctile(f"A2_{p}", [128, 2 * BO2], FP8) for p in range(NAB)]
            # A3 {0,1}: pitch 48; +1-col block at BO3
            A3 = [ctile(f"A3_{p}", [128, 2 * S3A], FP8) for p in range(NAB)]
            # A4 {0,1}: block0 = ch 0-127; block1 (offset NPOS) = ch 128-191
            # on partitions 0-63, zeros above
            A4 = [ctile(f"A4_{p}", [128, 2 * NPOS], FP8) for p in range(NAB)]
            MACCa = ctile("MACCa", [128, 2 * B], F32)
            MACCb = ctile("MACCb", [64, 2 * B], F32)
            Msum = ctile("Msum", [128, B], F32)
            MsumB = ctile("MsumB", [65, B], F32)

            # pad fill once; interiors are rewritten every image.
            for p in range(NA1):
                nc.gpsimd.memset(A1[p][:], 0.0)
            for p in range(NAB):
                nc.gpsimd.memset(A2[p][:], -1.0)
                nc.gpsimd.memset(A3[p][:], 0.0)
                nc.gpsimd.memset(A4[p][64:128, NPOS:2 * NPOS], 0.0)
            nc.vector.memset(MsumB[64:65, :], 1.0)

            w2a = cW2A[:].rearrange("p (j m) -> p j m", j=2)
            w2as = cW2As[:].rearrange("p (j m) -> p j m", j=2)
            w2b = cW2B[:].rearrange("p (j m) -> p j m", j=2)
            w2bs = cW2Bs[:].rearrange("p (j m) -> p j m", j=2)

            # ------ conv1 + binarize -> A1 (4 images per PE slot) ------
            def conv1_group(g):
                pq = g % 2
                nc.sync.dma_start(IC[pq][0:54, :], dX[2 * g])
                nc.sync.dma_start(IC[pq][64:118, :], dX[2 * g + 1])
                imgs = [4 * g + k for k in range(4)]
                bufs = [A1[im % NA1] for im in imgs]
                for r in range(8):
                    pss = []
                    for h in range(2):
                        o = h * 64
                        ps = pp.tile([128, 512], F32, tag="mm",
                                     name=f"ps_c1_{g}_{r}_{h}")
                        nc.tensor.matmul(
                            ps[:], cW1T[o:o + 54, :],
                            IC[pq][o:o + 54, r * 512:(r + 1) * 512],
                            start=True, stop=True)
                        pss.append(ps)
                    for h in range(2):
                        for par in range(2):
                            a1t = bufs[2 * h + par]
                            lo = par * 64
                            a1vv = a1t[lo:lo + 64, 0:S2A].rearrange(
                                "p (a b) -> p a b", b=PA)
                            nc.vector.tensor_scalar(
                                a1vv[:, r * 8 + 1:r * 8 + 9, 1:65],
                                pss[h][par * 64:par * 64 + 64, :].rearrange(
                                    "q (a b) -> q a b", b=64),
                                cT1[par * 64:par * 64 + 64], None, OP.is_gt)
                for k in range(4):
                    a1t = bufs[k]
                    par = k % 2
                    lo, hi = par * 64, 64 - par * 64
                    # +1-row copy on the other partition half
                    nc.sync.dma_start(a1t[hi:hi + 64, 0:S2A - PA],
                                      a1t[lo:lo + 64, PA:S2A])
                    # +1-col block (both halves)
                    nc.sync.dma_start(a1t[:, BO1:BO1 + S1A - 1],
                                      a1t[:, 1:S1A])

            # ---------------- layer bodies ----------------
            def l2_block(i):
                p = i % NAB
                par = i % 2
                a1f = A1[i % NA1][:, :]
                wa = w2a if par == 0 else w2as
                wb = w2b if par == 0 else w2bs
                lo = par * 64
                hi = 64 - lo
                a2vv = A2[p][:, 0:S2A].rearrange("p (a b) -> p a b", b=PA)
                for cpair in range(4):
                    c0, c1 = 2 * cpair, 2 * cpair + 1
                    pss = {}
                    for c in (c0, c1):
                        q0 = (c * 8) * PA
                        ps = pp.tile([128, 512], F32, tag="mm",
                                     name=f"ps_l2_{i}_{c}")
                        psv = ps[:].rearrange("q (a b) -> q a b", b=64)
                        pss[c] = (psv, q0)
                        # MM1: (0,0),(1,0) | +1col: (0,1),(1,1)
                        nc.tensor.matmul(
                            psv, wa,
                            _ap(a1f, q0, [[BO1, 2], [PA, 8], [1, 64]]),
                            start=True, stop=False, perf_mode=DR)
                        # MM2: (0,2),(1,2) | +2rows: (2,2),(zero)
                        nc.tensor.matmul(
                            psv, wb,
                            _ap(a1f, q0 + 2, [[160, 2], [PA, 8], [1, 64]]),
                            start=False, stop=False, perf_mode=DR)
                    # leftover {(2,0),(2,1)}: K=64 units, row-tiled pair
                    nc.tensor.matmul(
                        pss[c0][0], cW2C[lo:lo + 64, :].rearrange(
                            "p (j m) -> p j m", j=2),
                        _ap(a1f[lo:lo + 64, :], pss[c0][1] + 160,
                            [[BO1, 2], [PA, 8], [1, 64]]),
                        start=False, stop=True, perf_mode=DR)
                    nc.tensor.matmul(
                        pss[c1][0], cW2C[hi:hi + 64, :].rearrange(
                            "p (j m) -> p j m", j=2),
                        _ap(a1f[hi:hi + 64, :], pss[c1][1] + PA,
                            [[BO1, 2], [PA, 8], [1, 64]]),
                        start=False, stop=True, perf_mode=DR)
                    for c in (c0, c1):
                        y0 = c * 8
                        nc.scalar.activation(
                            a2vv[:, y0 + 1:y0 + 9, 1:65], pss[c][0],
                            ACT.Sign, bias=cT2[:], scale=1.0)
                    # per-pair +1-col block copy (rows 16*cpair+1 .. +16)
                    ql = (16 * cpair + 1) * PA
                    qh = (16 * cpair + 16) * PA + 66
                    nc.sync.dma_start(
                        A2[p][:, BO2 + ql - 1:BO2 + qh - 1],
                        A2[p][:, ql:qh])

            def l3_block(i):
                # row-split: each full-K matmul runs as two concurrent K=64
                # row-tiled units (ci halves) into two psum tiles; ACT
                # negates one into SBUF, fused DVE is_gt binarizes.
                p = i % NAB
                a2f = A2[p][:, :]
                a3v = A3[p][:, 0:S3A].rearrange("p (a b) -> p a b", b=P3A)
                for r in range(2):
                    y0 = r * 16
                    base = (2 * y0) * PA
                    ps = [pp.tile([128, 512], F32, tag="mm",
                                  name=f"ps_l3_{i}_{r}_{h}")
                          for h in range(2)]
                    for kx in range(3):
                        for h in range(2):
                            o = h * 64
                            nc.tensor.matmul(
                                ps[h][:],
                                cW3D[o:o + 64, kx * 256:(kx + 1) * 256]
                                .rearrange("p (j m) -> p j m", j=2),
                                _ap(a2f[o:o + 64, :], base + kx,
                                    [[PA, 2], [2 * PA, 16], [2, 32]]),
                                start=(kx == 0), stop=False, perf_mode=DR)
                    for h in range(2):
                        o = h * 64
                        nc.tensor.matmul(
                            ps[h][:],
                            cW3D3[o:o + 64, :].rearrange(
                                "p (j m) -> p j m", j=2),
                            _ap(a2f[o:o + 64, :], base + 2 * PA,
                                [[BO2, 2], [2 * PA, 16], [2, 32]]),
                            start=False, stop=False, perf_mode=DR)
                    for h in range(2):
                        o = h * 64
                        nc.tensor.matmul(
                            ps[h][:],
                            cW3S[o:o + 64, :].rearrange(
                                "p (j m) -> p j m", j=2),
                            _ap(a2f[o:o + 64, :], base + 2 * PA + 2,
                                [[16, 2], [2 * PA, 16], [2, 32]]),
                            start=False, stop=True, perf_mode=DR)
                    scr3 = wp.tile([128, 512], F32, tag="scr3",
                                   name=f"scr3_{i}_{r}")
                    nc.scalar.activation(scr3[:], ps[1][:], ACT.Copy,
                                         bias=0.0, scale=-1.0)
                    nc.vector.scalar_tensor_tensor(
                        a3v[:, y0 + 1:y0 + 17, 1:33],
                        ps[0][:].rearrange("q (a b) -> q a b", b=32),
                        cT3[:],
                        scr3[:].rearrange("q (a b) -> q a b", b=32),
                        OP.subtract, OP.is_gt)
                # +1-col block (one DMA per image)
                nc.sync.dma_start(
                    A3[p][:, BO3:BO3 + S3A - 1], A3[p][:, 1:S3A])

            def l4_block(i):
                # row-split: each full-K matmul runs as two concurrent K=64
                # row-tiled units (ci halves) into two psum tiles; DVE adds
                # them, ACT Sign binarizes (A4 is {-1,+1} here).
                p = i % NAB
                a3f = A3[p][:, :]
                a4av = A4[p][:, 0:NPOS].rearrange("p (a b) -> p a b", b=32)
                a4bv = A4[p][0:64, NPOS:2 * NPOS].rearrange(
                    "p (a b) -> p a b", b=32)
                for ci in range(2):
                    y0 = ci * 16
                    q0 = y0 * P3A
                    psa = [pp.tile([128, 512], F32, tag="mm",
                                   name=f"ps_l4a_{i}_{ci}_{h}")
                           for h in range(2)]
                    psb = [pp.tile([64, 512], F32, tag="mb2", bufs=2,
                                   name=f"ps_l4b_{i}_{ci}_{h}")
                           for h in range(2)]
                    for mb in range(2):
                        psx = psa if mb == 0 else psb
                        wd = cW4DA if mb == 0 else cW4DB
                        wd3 = cW4D3A if mb == 0 else cW4D3B
                        ws = cW4SA if mb == 0 else cW4SB
                        mw = 128 if mb == 0 else 64
                        for kx in range(3):
                            for h in range(2):
                                o = h * 64
                                nc.tensor.matmul(
                                    psx[h][:],
                                    wd[o:o + 64,
                                       kx * 2 * mw:(kx + 1) * 2 * mw]
                                    .rearrange("p (j m) -> p j m", j=2),
                                    _ap(a3f[o:o + 64, :], q0 + kx,
                                        [[P3A, 2], [P3A, 16], [1, 32]]),
                                    start=(kx == 0), stop=False,
                                    perf_mode=DR)
                        for h in range(2):
                            o = h * 64
                            nc.tensor.matmul(
                                psx[h][:],
                                wd3[o:o + 64, 0:2 * mw].rearrange(
                                    "p (j m) -> p j m", j=2),
                                _ap(a3f[o:o + 64, :], q0 + 2 * P3A,
                                    [[BO3, 2], [P3A, 16], [1, 32]]),
                                start=False, stop=False, perf_mode=DR)
                        for h in range(2):
                            o = h * 64
                            nc.tensor.matmul(
                                psx[h][:],
                                ws[o:o + 64, 0:2 * mw].rearrange(
                                    "p (j m) -> p j m", j=2),
                                _ap(a3f[o:o + 64, :], q0 + 2 * P3A + 2,
                                    [[16, 2], [P3A, 16], [1, 32]]),
                                start=False, stop=True, perf_mode=DR)
                    # combine ci halves: negate one psum into SBUF on
                    # ACT, then fused (psa0 - t4) is_gt (-psa1) on DVE
                    scr4a = wp.tile([128, 512], F32, tag="scr4a",
                                    name=f"scr4a_{i}_{ci}")
                    scr4b = wp.tile([64, 512], F32, tag="scr4b",
                                    name=f"scr4b_{i}_{ci}")
                    nc.scalar.activation(scr4a[:], psa[1][:], ACT.Copy,
                                         bias=0.0, scale=-1.0)
                    nc.scalar.activation(scr4b[:], psb[1][:], ACT.Copy,
                                         bias=0.0, scale=-1.0)
                    nc.vector.scalar_tensor_tensor(
                        a4av[:, y0:y0 + 16, 0:32],
                        psa[0][:].rearrange("q (a b) -> q a b", b=32),
                        cT4a[:],
                        scr4a[:].rearrange("q (a b) -> q a b", b=32),
                        OP.subtract, OP.is_gt)
                    nc.vector.scalar_tensor_tensor(
                        a4bv[:, y0:y0 + 16, 0:32],
                        psb[0][:].rearrange("q (a b) -> q a b", b=32),
                        cT4b[0:64],
                        scr4b[:].rearrange("q (a b) -> q a b", b=32),
                        OP.subtract, OP.is_gt)

            def l5_block(i):
                p = i % NAB
                a4f = A4[p][:, :]
                for c in range(2):
                    psa = pp.tile([128, 512], F32, tag="mm",
                                  name=f"ps_l5a_{i}_{c}")
                    psb = pp.tile([64, 512], F32, tag="mb2", bufs=2,
                                  name=f"ps_l5b_{i}_{c}")
                    rhs = _ap(a4f, c * 512, [[NPOS, 2], [1, 512]])
                    nc.tensor.matmul(
                        psa[:], cW5DA[:].rearrange("p (j m) -> p j m", j=2),
                        rhs, start=True, stop=True, perf_mode=DR)
                    nc.tensor.matmul(
                        psb[:], cW5DB[:].rearrange("p (j m) -> p j m", j=2),
                        rhs, start=True, stop=True, perf_mode=DR)
                    scra = wp.tile([128, 512], F32, tag="scr_a",
                                   name=f"scra_{i}_{c}")
                    scrb = wp.tile([64, 512], F32, tag="scr_b",
                                   name=f"scrb_{i}_{c}")
                    nc.scalar.activation(
                        scra[:], psa[:], ACT.Relu, bias=cB5a[:], scale=cA5a[:],
                        accum_out=MACCa[:, 2 * i + c: 2 * i + c + 1])
                    nc.scalar.activation(
                        scrb[:], psb[:], ACT.Relu, bias=cB5b[:], scale=cA5b[:],
                        accum_out=MACCb[:, 2 * i + c: 2 * i + c + 1])

            # ---------------- main pipeline ----------------
            START = {1: 1, 5: 2, 9: 3}
            for _rep in range(reps):
                conv1_group(0)
                for i in range(B):
                    if i in START:
                        conv1_group(START[i])
                    l2_block(i)
                    l3_block(i)
                    l4_block(i)
                    l5_block(i)

            # ---------------- GAP/FC/softmax tail ----------------
            nc.vector.tensor_reduce(
                Msum[:, 0:B], MACCa[:].rearrange("p (i c) -> p i c", c=2),
                axis=AX.X, op=OP.add)
            nc.vector.tensor_reduce(
                MsumB[0:64, 0:B], MACCb[:].rearrange("p (i c) -> p i c", c=2),
                axis=AX.X, op=OP.add)

            psf = pp.tile([64, 512], F32, tag="mb2", bufs=2, name="ps_fc")
            nc.tensor.matmul(psf[0:16, 0:12], Msum[:, 0:B], cWTa[:],
                             start=True, stop=False)
            nc.tensor.matmul(psf[0:16, 0:12], MsumB[:, 0:B], cWTb[:],
                             start=False, stop=True)

            negmax = cp.tile([16, 1], F32, tag="negmax", name="negmax")
            esum = cp.tile([16, 1], F32, tag="esum", name="esum")
            rsum = cp.tile([16, 1], F32, tag="rsum", name="rsum")
            etile = cp.tile([16, 12], F32, tag="etile", name="etile")
            yout = cp.tile([16, 12], F32, tag="yout", name="yout")

            nc.vector.tensor_reduce(negmax[:], psf[0:16, 0:12], axis=AX.X,
                                    op=OP.max, negate=True)
            nc.scalar.activation(etile[:], psf[0:16, 0:12], ACT.Exp,
                                 bias=negmax[:], scale=1.0, accum_out=esum[:])
            nc.vector.reciprocal(rsum[:], esum[:])
            nc.vector.tensor_scalar(yout[:], etile[:], rsum[:], None, OP.mult)
            nc.sync.dma_start(dY[:], yout[:])

    nc.compile()
    _CACHE[key] = nc
    return _CACHE


def _host_prep(inputs):
    """Fold BN into thresholds/affines; sign-binarize weights; build per-core
    input maps."""
    f32 = np.float32
    fp8 = mybir.dt.np(FP8)

    x = np.asarray(inputs["x"], f32)

    def inv(l):
        return (np.asarray(inputs[f"bn{l}_g"], f32)
                / np.sqrt(np.asarray(inputs[f"bn{l}_v"], f32)
                          + np.float32(EPS)))

    invs = {l: inv(l) for l in (1, 2, 3, 4, 5)}
    for l in (1, 2, 3, 4):
        assert (invs[l] > 0).all(), f"bn{l} scale not positive"

    def thr(l):
        return (np.asarray(inputs[f"bn{l}_m"], f32)
                - np.asarray(inputs[f"bn{l}_b"], f32) / invs[l])

    sw2 = np.sign(np.asarray(inputs["w2"], f32))       # [128,64,3,3]
    sw3 = np.sign(np.asarray(inputs["w3"], f32))       # [128,128,3,3]
    sw4 = np.sign(np.asarray(inputs["w4"], f32))       # [192,128,3,3]
    sw5 = np.sign(np.asarray(inputs["w5"], f32))       # [192,192,1,1]

    t1v = thr(1) - np.asarray(inputs["conv1_b"], f32)
    t1 = np.concatenate([t1v, t1v]).reshape(128, 1)
    # A1 {0,1}; A2 {-1,+1} via ScalarE Sign (bias = -thr2)
    t2 = (-thr(2)).reshape(128, 1)
    # L3 consumes pm1 A2, emits {0,1} A3 via is_gt with folded threshold
    sw3sum = sw3.sum(axis=(1, 2, 3))
    t3 = (2.0 * thr(3) - sw3sum).reshape(128, 1)
    # L4 consumes {0,1} A3 -> plain thresholds; A4 stays {0,1}
    t4 = thr(4)
    a5 = invs[5]
    b5 = (np.asarray(inputs["bn5_b"], f32)
          - np.asarray(inputs["bn5_m"], f32) * invs[5])

    # conv1 weights: block-diag [54, 128] replicated at rows 0/64.
    w1 = np.asarray(inputs["conv1_w"], f32)           # [64,1,3,3]
    w1t = np.ascontiguousarray(w1[:, 0].reshape(64, 9).T)  # [9, 64]
    w1hi = w1t.astype(np.float16)
    w1lo = (w1t - w1hi.astype(f32)).astype(np.float16)
    w1t27 = np.concatenate(
        [w1hi, w1lo, (w1hi.astype(f32) / 64.0).astype(np.float16)], axis=0)
    w1t4 = np.zeros((128, 128), np.float16)
    w1t4[0:27, 0:64] = w1t27
    w1t4[27:54, 64:128] = w1t27
    w1t4[64:91, 0:64] = w1t27
    w1t4[91:118, 64:128] = w1t27

    # L2 packs (normal layout: partitions 0-63 = unshifted, 64-127 = +1 row)
    w2a_ = np.zeros((128, 2, 128), f32)
    w2a_[0:64, 0] = sw2[:, :, 0, 0].T
    w2a_[64:128, 0] = sw2[:, :, 1, 0].T
    w2a_[0:64, 1] = sw2[:, :, 0, 1].T
    w2a_[64:128, 1] = sw2[:, :, 1, 1].T
    w2b_ = np.zeros((128, 2, 128), f32)
    w2b_[0:64, 0] = sw2[:, :, 0, 2].T
    w2b_[64:128, 0] = sw2[:, :, 1, 2].T
    w2b_[0:64, 1] = sw2[:, :, 2, 2].T
    w2c_ = np.zeros((128, 2, 128), f32)
    w2c_[0:64, 0] = sw2[:, :, 2, 0].T
    w2c_[0:64, 1] = sw2[:, :, 2, 1].T
    w2c_[64:128, 0] = sw2[:, :, 2, 0].T
    w2c_[64:128, 1] = sw2[:, :, 2, 1].T
    w2as_ = np.concatenate([w2a_[64:128], w2a_[0:64]], axis=0)
    w2bs_ = np.concatenate([w2b_[64:128], w2b_[0:64]], axis=0)

    # L3 packs: w3d[kx]: j = ky in {0,1}; w3d3: j0=(2,0), j1=(2,1); w3s solo
    w3d = np.zeros((128, 3, 2, 128), f32)
    for kx in range(3):
        for j in range(2):
            w3d[:, kx, j] = sw3[:, :, j, kx].T
    w3d3 = np.zeros((128, 2, 128), f32)
    w3d3[:, 0] = sw3[:, :, 2, 0].T
    w3d3[:, 1] = sw3[:, :, 2, 1].T
    w3s = np.concatenate([sw3[:, :, 2, 2].T, np.zeros((128, 128), f32)],
                         axis=1)

    # L4 packs (baseline layout)
    w4da = np.zeros((128, 3, 2, 128), f32)
    w4db = np.zeros((128, 3, 2, 64), f32)
    for kx in range(3):
        for j in range(2):
            w4da[:, kx, j] = sw4[:128, :, j, kx].T
            w4db[:, kx, j] = sw4[128:, :, j, kx].T
    w4d3a = np.zeros((128, 2, 128), f32)
    w4d3b = np.zeros((128, 2, 64), f32)
    for j in range(2):
        w4d3a[:, j] = sw4[:128, :, 2, j].T
        w4d3b[:, j] = sw4[128:, :, 2, j].T
    w4sa = np.concatenate([sw4[:128, :, 2, 2].T, np.zeros((128, 128), f32)],
                          axis=1)
    w4sb = np.concatenate([sw4[128:, :, 2, 2].T, np.zeros((128, 64), f32)],
                          axis=1)

    # L5 packs (baseline layout)
    w5 = sw5[:, :, 0, 0]
    w5da = np.zeros((128, 2, 128), f32)
    w5da[:, 0] = w5[:128, :128].T
    w5da[0:64, 1] = w5[:128, 128:].T
    w5db = np.zeros((128, 2, 64), f32)
    w5db[:, 0] = w5[128:, :128].T
    w5db[0:64, 1] = w5[128:, 128:].T

    fc_w = np.asarray(inputs["fc_w"], f32)
    c6w = np.asarray(inputs["conv6_w"], f32)[:, :, 0, 0]
    Wp = (fc_w @ c6w) / np.float32(NPOS)
    cvec = fc_w @ np.asarray(inputs["conv6_b"], f32) + np.asarray(
        inputs["fc_b"], f32)
    wta = np.ascontiguousarray(Wp[:, :128].T)
    wtb = np.zeros((65, 12), f32)
    wtb[:64] = Wp[:, 128:].T
    wtb[64] = cvec

    shared = {
        "w1t": w1t4,
        "w2a": w2a_.reshape(128, 256).astype(fp8),
        "w2as": w2as_.reshape(128, 256).astype(fp8),
        "w2b": w2b_.reshape(128, 256).astype(fp8),
        "w2bs": w2bs_.reshape(128, 256).astype(fp8),
        "w2c": w2c_.reshape(128, 256).astype(fp8),
        "w3d": w3d.reshape(128, 768).astype(fp8),
        "w3d3": w3d3.reshape(128, 256).astype(fp8),
        "w3s": w3s.astype(fp8),
        "w4da": w4da.reshape(128, 768).astype(fp8),
        "w4d3a": w4d3a.reshape(128, 256).astype(fp8),
        "w4sa": w4sa.astype(fp8),
        "w4db": w4db.reshape(128, 384).astype(fp8),
        "w4d3b": w4d3b.reshape(128, 128).astype(fp8),
        "w4sb": w4sb.astype(fp8),
        "w5da": w5da.reshape(128, 256).astype(fp8),
        "w5db": w5db.reshape(128, 128).astype(fp8),
        "t1": t1.astype(f32), "t2": t2.astype(f32), "t3": t3.astype(f32),
        "t4a": t4[:128].reshape(128, 1).astype(f32),
        "t4b": np.concatenate([t4[128:], t4[128:]]).reshape(128, 1)
        .astype(f32),
        "a5a": a5[:128].reshape(128, 1).astype(f32),
        "a5b": a5[128:].reshape(64, 1).astype(f32),
        "b5a": b5[:128].reshape(128, 1).astype(f32),
        "b5b": b5[128:].reshape(64, 1).astype(f32),
        "wta": wta.astype(f32), "wtb": wtb.astype(f32),
    }
    # host im2col: cols[b, 3*ky+kx, y*64+x] = xpad[b, 2y+ky, 2x+kx]
    xpad = np.pad(x[:, 0], ((0, 0), (1, 1), (1, 1)))
    cols = np.stack([xpad[:, ky:ky + 127:2, kx:kx + 127:2]
                     for ky in range(3) for kx in range(3)],
                    axis=1).reshape(x.shape[0], 9, 4096)
    chi = cols.astype(np.float16)
    clo = ((cols - chi.astype(f32)) * 64.0).astype(np.float16)
    cols27 = np.concatenate([chi, chi, clo], axis=1)    # [128, 27, 4096]
    cols54 = np.ascontiguousarray(
        cols27.reshape(x.shape[0] // 2, 54, 4096))      # image pairs
    in_maps = []
    for c in range(N_CORES):
        m = dict(shared)
        m["x"] = np.ascontiguousarray(cols54[c * (B // 2):(c + 1) * (B // 2)])
        in_maps.append(m)
    return in_maps


def kernel(**inputs):
    cache = _build()
    in_maps = _host_prep(inputs)
    res = run_bass_kernel_spmd(cache["nc1"], in_maps,
                               core_ids=list(range(N_CORES)))
    _CACHE["last_results"] = res
    return np.concatenate([res.results[c]["y"] for c in range(N_CORES)],
                          axis=0)


# ---------------------------------------------------------------------------
# numpy golden model of the device algorithm (validates packing w/o HW)
# ---------------------------------------------------------------------------
def golden(inputs):
    f32 = np.float32
    in_maps = _host_prep(inputs)
    outs = []
    for m in in_maps:
        cols = np.asarray(m["x"], f32).reshape(B, 27, 4096)
        t1 = m["t1"][:64, 0]
        w1t = np.asarray(m["w1t"][0:27, 0:64], f32)
        c1 = np.einsum("btn,tc->bcn", cols, w1t).reshape(-1, 64, 64, 64)
        a1 = (c1 > t1[None, :, None, None]).astype(f32)

        def bconv(a, wt, stride, pad_val=0.0):
            Bn, C, H, W = a.shape
            ap = np.pad(a, ((0, 0), (0, 0), (1, 1), (1, 1)),
                        constant_values=pad_val)
            Ho, Wo = H // stride, W // stride
            out = np.zeros((Bn, wt.shape[2], Ho, Wo), f32)
            for t in range(9):
                ky, kx = t // 3, t % 3
                sl = ap[:, :, ky:ky + H:stride, kx:kx + W:stride][
                    :, :, :Ho, :Wo]
                out += np.einsum("bcyx,cd->bdyx", sl, wt[:, t])
            return out

        # reconstruct w2 [ci, t, co] from packs (normal layout)
        w2a_ = np.asarray(m["w2a"], f32).reshape(128, 2, 128)
        w2b_ = np.asarray(m["w2b"], f32).reshape(128, 2, 128)
        w2c_ = np.asarray(m["w2c"], f32).reshape(128, 2, 128)
        w2 = np.zeros((64, 9, 128), f32)
        w2[:, 0] = w2a_[0:64, 0]     # (0,0)
        w2[:, 3] = w2a_[64:128, 0]   # (1,0)
        w2[:, 1] = w2a_[0:64, 1]     # (0,1)
        w2[:, 4] = w2a_[64:128, 1]   # (1,1)
        w2[:, 2] = w2b_[0:64, 0]     # (0,2)
        w2[:, 5] = w2b_[64:128, 0]   # (1,2)
        w2[:, 8] = w2b_[0:64, 1]     # (2,2)
        w2[:, 6] = w2c_[0:64, 0]     # (2,0)
        w2[:, 7] = w2c_[0:64, 1]     # (2,1)
        c2 = bconv(a1, w2, 1)
        a2 = np.sign(c2 + m["t2"].reshape(1, 128, 1, 1)).astype(f32)

        w3dg = np.asarray(m["w3d"], f32).reshape(128, 3, 2, 128)
        w3d3g = np.asarray(m["w3d3"], f32).reshape(128, 2, 128)
        w3 = np.zeros((128, 9, 128), f32)
        for kx in range(3):
            for j in range(2):
                w3[:, 3 * j + kx] = w3dg[:, kx, j]
        w3[:, 6] = w3d3g[:, 0]
        w3[:, 7] = w3d3g[:, 1]
        w3[:, 8] = np.asarray(m["w3s"], f32)[:, :128]
        c3 = bconv(a2, w3, 2, pad_val=-1.0)
        a3 = (c3 > m["t3"].reshape(1, 128, 1, 1)).astype(f32)

        w4da = np.asarray(m["w4da"], f32).reshape(128, 3, 2, 128)
        w4db = np.asarray(m["w4db"], f32).reshape(128, 3, 2, 64)
        w4d3a = np.asarray(m["w4d3a"], f32).reshape(128, 2, 128)
        w4d3b = np.asarray(m["w4d3b"], f32).reshape(128, 2, 64)
        w4 = np.zeros((128, 9, 192), f32)
        for kx in range(3):
            for j in range(2):
                w4[:, 3 * j + kx, :128] = w4da[:, kx, j]
                w4[:, 3 * j + kx, 128:] = w4db[:, kx, j]
        for j in range(2):
            w4[:, 6 + j, :128] = w4d3a[:, j]
            w4[:, 6 + j, 128:] = w4d3b[:, j]
        w4[:, 8, :128] = np.asarray(m["w4sa"], f32)[:, :128]
        w4[:, 8, 128:] = np.asarray(m["w4sb"], f32)[:, :64]
        c4 = bconv(a3, w4, 1, pad_val=0.0)
        a4 = np.concatenate([
            (c4[:, :128] > m["t4a"].reshape(1, 128, 1, 1)).astype(f32),
            (c4[:, 128:] > m["t4b"][:64].reshape(1, 64, 1, 1)).astype(f32)],
            axis=1)

        w5dag = np.asarray(m["w5da"], f32).reshape(128, 2, 128)
        w5dbg = np.asarray(m["w5db"], f32).reshape(128, 2, 64)
        w5 = np.zeros((192, 192), f32)
        w5[:128, :128] = w5dag[:, 0]
        w5[128:, :128] = w5dag[0:64, 1]
        w5[:128, 128:] = w5dbg[:, 0]
        w5[128:, 128:] = w5dbg[0:64, 1]
        c5 = np.einsum("bcyx,cd->bdyx", a4, w5)
        a5v = np.concatenate([m["a5a"], m["a5b"]], axis=0).reshape(
            1, 192, 1, 1)
        b5v = np.concatenate([m["b5a"], m["b5b"]], axis=0).reshape(
            1, 192, 1, 1)
        h5 = np.maximum(a5v * c5 + b5v, 0.0)
        sums = h5.sum(axis=(2, 3))
        WT = np.concatenate([m["wta"], m["wtb"][:64]], axis=0)
        logits = sums @ WT + m["wtb"][64][None, :]
        z = logits - logits.max(axis=1, keepdims=True)
        e = np.exp(z)
        outs.append(e / e.sum(axis=1, keepdims=True))
    return np.concatenate(outs, axis=0)
